# revision 18
# baseline (speedup 1.0000x reference)
"""Trainium2 Bass kernel for sparse (shared-prefix) GQA decode attention.

Full-input contract: kernel(**inputs) takes the unsharded tensors from
setup_inputs() and returns the full [16, 1, 4096] float32 output.

Sharding: tensor-parallel over heads across 8 NeuronCores. Core m owns
query heads 4m..4m+3 and kv head m (GQA group m), i.e. wq columns
[512m, 512m+512), wk/wv columns [128m, 128m+128), wo rows [512m, 512m+512),
and head m of the kv caches. Each core computes a partial output
y_m = attn_m @ wo_m; the host sums the 8 partials (the "all-reduce").

Device-side layout: scores are kept transposed, sT[j, r] with r = 4b+h on
the free dim, so every engine op starts at partition 0 (the hardware only
allows aligned partition bases) and the probabilities come out already in
the orientation the PV matmul needs.

Problem constants (hardcoded per the harness contract): bsz=16, seqlen=1,
dim=4096, n_heads=32, n_kv=8, hd=128, start_pos=2048,
shared_prefix_length=512 -> rsp=1536, L=2049.
"""

import math
import os
import sys
import types

import numpy as np

# ----------------------------------------------------------------------------
# environment patches (self-contained; no /root/problem reads)
# ----------------------------------------------------------------------------


def _patch_tile_drain():
    """The stock TileContext._drain_and_barrier puts one sem-wait per live
    semaphore on a single Drain instruction; the walrus build in this image
    only accepts a single sync wait per instruction ("Too many sync wait
    commands"). Re-emit the waits as individual EventSemaphore instructions
    on the same sequencer instead."""
    import concourse.tile as tile
    from concourse.vector_clock import ScopedClock

    if getattr(tile.TileContext, "_drain_patched", False):
        return

    def _drain_and_barrier(self, tick_clock, wait_clock):
        nc = self.nc
        drain_inst = nc.sync.drain()
        wait_clock.add_sem_waits(
            drain_inst.ins, ScopedClock({None: tick_clock.global_clock})
        )
        waits = list(drain_inst.ins.sync_info.on_wait)
        if len(waits) > 1:
            by_name = {h.name: h for h in self.sems.allocated().values()}
            try:
                drain_inst.ins.sync_info = None
            except Exception:
                pass
            for w in waits:
                h = by_name.get(w.ant_name)
                assert h is not None, f"no handle for sem {w.ant_name}"
                nc.sync.wait_ge(h, w.wait_value)

        # No barrier / explicit sem clears: every instruction transitively
        # precedes the SP wait chain above, and the NRT postamble already
        # resets all semaphores. Only do the python-side bookkeeping.
        assert self.sems is not None
        popped = nc._tile_sem_poison_stack.pop()
        assert popped is self._sem_poison
        nums = [h.num for h in self.sems.allocated().values()]
        nc._state.prepend_free_semaphores(nums)
        for ps in nc._tile_sem_poison_stack:
            ps.update(nums)

    tile.TileContext._drain_and_barrier = _drain_and_barrier
    tile.TileContext._drain_patched = True


def _install_ntff_hook():
    """Optional: register the axon NTFF profile hook (missing from the
    trimmed antenv package) so trace=True works for profiling, and stub the
    S3 artifact upload (zero-egress container)."""
    try:
        if "antenv.axon_hooks" not in sys.modules:
            mod = types.ModuleType("antenv.axon_hooks")
            mod._hook = None
            mod.set_axon_ntff_profile_hook = lambda h: setattr(mod, "_hook", h)
            mod.get_axon_ntff_profile_hook = lambda: mod._hook
            sys.modules["antenv.axon_hooks"] = mod
            import antenv

            antenv.axon_hooks = mod
            from trn_agent_boot.trn_boot import _ntff_profile_via_ctypes

            mod.set_axon_ntff_profile_hook(
                _ntff_profile_via_ctypes("/opt/axon/libaxon_pjrt.so")
            )
        import concourse.bass_utils as bu

        bu.upload_artifacts = lambda tmpdir: tmpdir
    except Exception:
        pass




def _legalize_multiwait(nc, max_waits=1):
    """This walrus build accepts at most one sync wait per instruction.
    Hoist excess waits into standalone single-wait EventSemaphore
    instructions inserted immediately before, on the same engine."""
    import bass_rust

    uid = 0
    for f in nc.m.functions:
        for bb in f.blocks:
            insts = list(bb.instructions)
            out = []
            changed = False
            for ins in insts:
                si = ins.sync_info
                if si is not None:
                    waits = list(si.on_wait)
                    if len(waits) > max_waits:
                        for w in waits[:-max_waits]:
                            ev = bass_rust.InstEventSemaphore(
                                name=f"{ins.name}_xw{uid}"
                            )
                            uid += 1
                            ev.engine = ins.engine
                            ev.sync_info = bass_rust.SyncInfo(
                                on_wait=[w], on_update=[]
                            )
                            out.append(ev)
                        ins.sync_info = bass_rust.SyncInfo(
                            on_wait=waits[-max_waits:],
                            on_update=list(si.on_update),
                        )
                        changed = True
                out.append(ins)
            if changed:
                bb.instructions = out


# ----------------------------------------------------------------------------
# constants
# ----------------------------------------------------------------------------

N_CORES = 8
B = 16            # batch
DIM = 4096
N_HEADS = 32
N_KV = 8
HD = 128
NH = N_HEADS // N_CORES      # 4 local q heads
R = B * NH                   # 64 (b,h) rows, r = 4*b + h
SOFTMAX_SCALE = 1.0 / math.sqrt(HD)
NEG_BIG = -1.0e30

# stream dtype for weights / kv-cache / matmul operands. "bfloat16" halves the
# HBM traffic (memory-bound kernel); softmax stays fp32 and all matmuls
# accumulate in fp32 PSUM.
STREAM_DTYPE = os.environ.get("KERNEL_STREAM_DTYPE", "bfloat16")
# kv-cache stream dtype: float8e3 (e3m4) halves the dominant HBM stream;
# scores/PV matmuls mix fp8 k/v with bf16 q/probs (PE allows mixed operands)
KV_DTYPE = os.environ.get("KERNEL_KV_DTYPE", "float8e3")
# use the fp32r (full-rate) matmul mode when streaming fp32
F32R = os.environ.get("KERNEL_F32R", "1") == "1"


# ----------------------------------------------------------------------------
# device kernel
# ----------------------------------------------------------------------------


def _build_nc(spl, rsp, dt_name, kv_dt_name):
    import concourse.bass as bass
    import concourse.tile as tile
    from concourse import mybir
    from concourse.masks import make_identity
    from concourse.mybir import ActivationFunctionType as AF

    DT = getattr(mybir.dt, dt_name)
    KVDT = getattr(mybir.dt, kv_dt_name)
    f32 = mybir.dt.float32
    assert spl % 128 == 0 and rsp % 512 == 0
    SH_CH = spl // 128          # shared j-chunks (4)
    BCH = rsp // 128            # per-batch j-chunks (12)
    NCH = SH_CH + BCH + 1       # total chunks incl. new-token chunk (17)
    NWQ = 8                     # wq split into 8 fine tiles (stream chasing)
    WQK = 32 // NWQ             # k-chunks per wq tile (4)
    NKG = 4                     # kv batch groups (4 batches each)
    SPIN = int(os.environ.get("KERNEL_SPIN", "56"))

    def mm(ap):
        if dt_name == "float32" and F32R:
            return ap.bitcast(mybir.dt.float32r)
        return ap

    nc = bass.Bass(
        "TRN2", target_bir_lowering=False, debug=False, num_devices=N_CORES
    )

    def din(name, shape, dt=DT):
        return nc.dram_tensor(name, shape, dt, kind="ExternalInput").ap()

    # byte-packed consts: xT (bf16, 1KB) | shared kT+v (fp8, 1KB) |
    # rope cos/sin + mask (f32, rows 0-15, 2304B)
    pack_d = din("pack", [128, 4864], mybir.dt.uint8)
    wq_d = din("wq", [NWQ, 128, WQK * 512])
    wkv_d = din("wkv", [128, 32 * 256], KVDT)
    kT_d = din("kT", [NKG, 128, 4 * rsp], KVDT)
    v_d = din("v", [NKG, 128, 4 * rsp], KVDT)
    wo_d = din("wo", [4, 128, 2 * 4 * 512])
    y_d = nc.dram_tensor("y", [B, DIM], f32, kind="ExternalOutput").ap()

    with tile.TileContext(nc) as tc:
        with tc.tile_pool(name="const", bufs=1) as const, \
             tc.tile_pool(name="wpool", bufs=NWQ) as wpool, \
             tc.tile_pool(name="kpool", bufs=NKG) as kpool, \
             tc.tile_pool(name="vpool", bufs=NKG) as vpool, \
             tc.tile_pool(name="wopool", bufs=4) as wopool, \
             tc.tile_pool(name="tmp", bufs=4) as tmp:

            # ---------------- resident tiles ----------------
            id_sb = const.tile([64, 64], DT)
            make_identity(nc, id_sb)
            ones_sb = const.tile([128, 1], DT)
            nc.vector.memset(ones_sb, 1.0)
            ones1p = const.tile([1, 128], DT)
            nc.vector.memset(ones1p, 1.0)
            ones1pf = const.tile([1, 128], f32)
            nc.vector.memset(ones1pf, 1.0)
            onescf = const.tile([16, 1], f32)
            nc.vector.memset(onescf, 1.0)
            zeros1p = const.tile([1, R], DT)
            nc.vector.memset(zeros1p, 0.0)

            pack_sb = const.tile([128, 4864], mybir.dt.uint8)
            nc.sync.dma_start(out=pack_sb, in_=pack_d)
            xT_sb = pack_sb[:, :1024].bitcast(DT)
            shkT_sb = pack_sb[:, 1024 : 1024 + spl].bitcast(KVDT)
            shv_sb = pack_sb[:, 1024 + spl : 2048].bitcast(KVDT)
            rp_sb = pack_sb[:B, 2048:].bitcast(f32)    # [16, 704]
            crep_sb = rp_sb[:, : NH * 64]
            srep_sb = rp_sb[:, NH * 64 : 2 * NH * 64]
            maskf_sb = rp_sb[:, 2 * NH * 64 : 2 * NH * 64 + 64]  # b==r//4
            ckrep_sb = rp_sb[:, 2 * NH * 64 + 64 : 2 * NH * 64 + 128]  # cos/64
            skrep_sb = rp_sb[:, 2 * NH * 64 + 128 :]                   # sin/64
            mask_bf = const.tile([B, 64], DT)
            nc.vector.tensor_copy(mask_bf, maskf_sb)

            qT_sb = const.tile([128, R], DT)        # cols r = 4b+h
            xkT_sb = const.tile([128, B], DT)
            xv_sb = const.tile([B, HD], DT)
            sT_sb = const.tile([128, NCH, R], f32)  # transposed scores
            pT_sb = const.tile([128, NCH, R], DT)   # transposed probabilities
            sum1_sb = const.tile([1, R], f32)
            rinv1_sb = const.tile([1, R], f32)
            rinv_bc = const.tile([128, R], DT)      # rinv broadcast, cols r
            attnT_sb = const.tile([128, R], DT)     # PV result, cols r
            attnTn_sb = const.tile([128, R], DT)    # normalized
            ntm_sb = const.tile([B, R], f32)        # masked new-token scores
            ntp_sb = const.tile([B, R], DT)         # masked new-token probs
            y_sb = const.tile([B, DIM], f32)

            # ---------------- PE p-state warmup spin ----------------
            if SPIN:
                with tc.tile_pool(name="pwarm", bufs=1, space="PSUM") as pw:
                    wps = pw.tile([64, 64], DT)
                    for _ in range(SPIN):
                        nc.tensor.transpose(wps, id_sb, id_sb)

            # ---------------- phase A: projections + rope ----------------
            with tc.tile_pool(name="psA", bufs=1, space="PSUM") as psA, \
                 tc.tile_pool(name="ptrA", bufs=2, space="PSUM") as ptrA:
                xq_ps = psA.tile([B, NH * HD], f32)
                for t in range(NWQ):
                    wt = wpool.tile([128, WQK * 512], DT, tag="wq", name="wt")
                    nc.sync.dma_start(out=wt, in_=wq_d[t])
                    for c in range(WQK):
                        k = WQK * t + c
                        nc.tensor.matmul(
                            xq_ps,
                            mm(xT_sb[:, B * k : B * (k + 1)]),
                            mm(wt[:, 512 * c : 512 * (c + 1)]),
                            start=(k == 0),
                            stop=(k == 31),
                        )
                xkv_ps = psA.tile([B, 2 * HD], f32)
                wkv_sb = const.tile([128, 32 * 256], KVDT)
                nc.sync.dma_start(out=wkv_sb, in_=wkv_d)
                for k in range(32):
                    nc.tensor.matmul(
                        xkv_ps,
                        mm(xT_sb[:, B * k : B * (k + 1)]),
                        mm(wkv_sb[:, 256 * k : 256 * (k + 1)]),
                        start=(k == 0),
                        stop=(k == 31),
                    )

                # rope: pairs (even, odd) along hd; cos/sin repeated per
                # head (k uses cos/64, sin/64 to descale the x64 fp8 wkv)
                def rope(dst, src_ps, width, c_ap=None, s_ap=None):
                    e = src_ps.rearrange("p (n two) -> p n two", two=2)[:, :, 0]
                    o = src_ps.rearrange("p (n two) -> p n two", two=2)[:, :, 1]
                    de = dst.rearrange("p (n two) -> p n two", two=2)[:, :, 0]
                    do = dst.rearrange("p (n two) -> p n two", two=2)[:, :, 1]
                    c_ap = crep_sb[:, :width] if c_ap is None else c_ap
                    s_ap = srep_sb[:, :width] if s_ap is None else s_ap
                    t1 = tmp.tile([B, NH * 64], f32, tag="t1", name="t1")[:, :width]
                    t2 = tmp.tile([B, NH * 64], f32, tag="t2", name="t2")[:, :width]
                    nc.vector.tensor_mul(t1, e, c_ap)
                    nc.vector.tensor_mul(t2, o, s_ap)
                    nc.vector.tensor_sub(de, t1, t2)
                    t3 = tmp.tile([B, NH * 64], f32, tag="t1", name="t3")[:, :width]
                    t4 = tmp.tile([B, NH * 64], f32, tag="t2", name="t4")[:, :width]
                    nc.vector.tensor_mul(t3, e, s_ap)
                    nc.vector.tensor_mul(t4, o, c_ap)
                    nc.vector.tensor_add(do, t3, t4)

                xq_r = const.tile([B, NH * HD], DT)
                rope(xq_r, xq_ps, NH * 64)
                xk_r = const.tile([B, HD], DT)
                rope(xk_r, xkv_ps[:, :HD], 64, ckrep_sb, skrep_sb)
                nc.scalar.activation(out=xv_sb, in_=xkv_ps[:, HD:],
                                     func=AF.Copy, scale=1.0 / 64.0)

                # qT (cols r = 4b+h) via per-head PE transposes
                for h in range(NH):
                    tp = ptrA.tile([128, B], DT, tag="tq", name="tp")
                    nc.tensor.transpose(
                        tp, xq_r[:, HD * h : HD * (h + 1)], id_sb[:B, :B]
                    )
                    out_ap = qT_sb.rearrange("p (b h) -> p b h", h=NH)[:, :, h]
                    nc.vector.tensor_copy(out_ap, tp)
                tpk = ptrA.tile([128, B], DT, tag="tq", name="tpk")
                nc.tensor.transpose(tpk, xk_r, id_sb[:B, :B])
                nc.vector.tensor_copy(xkT_sb, tpk)

            # ---------------- phase B: transposed scores ----------------
            # new-token chunk: partitions 1.. never written -> -inf
            nc.vector.memset(sT_sb[:, NCH - 1, :], NEG_BIG)

            kts = [
                kpool.tile([128, 4 * rsp], KVDT, tag="kt", name="kt")
                for _ in range(NKG)
            ]
            vts = [
                vpool.tile([128, 4 * rsp], KVDT, tag="vt", name="vt")
                for _ in range(NKG)
            ]
            wots = [
                wopool.tile([128, 2 * 4 * 512], DT, tag="wo", name="wot")
                for _ in range(4)
            ]
            # kv issue order interleaved so scores lead PV by one group,
            # then the wo stream last (needed only for the tail projection)
            for a, b in [(0, None), (1, 0), (2, 1), (3, 2), (None, 3)]:
                if a is not None:
                    nc.sync.dma_start(out=kts[a], in_=kT_d[a])
                if b is not None:
                    nc.sync.dma_start(out=vts[b], in_=v_d[b])
            for n in range(4):
                nc.sync.dma_start(out=wots[n], in_=wo_d[n])

            with tc.tile_pool(name="pqk", bufs=4, space="PSUM") as pqk, \
                 tc.tile_pool(name="ppv", bufs=1, space="PSUM") as ppv, \
                 tc.tile_pool(name="ps1", bufs=1, space="PSUM") as ps1, \
                 tc.tile_pool(name="paux", bufs=2, space="PSUM") as paux:

                # shared-prefix scores: all 64 q rows at once per j-chunk
                for c in range(SH_CH):
                    qs = pqk.tile([128, R], f32, tag="qkb", name="qs")
                    nc.tensor.matmul(
                        qs, mm(shkT_sb[:, 128 * c : 128 * (c + 1)]), mm(qT_sb),
                        start=True, stop=True,
                    )
                    nc.vector.tensor_copy(sT_sb[:, c, :], qs)

                # new-token scores, all batches in one masked matmul:
                # nt16[b, r] = xk_b . q_r ; keep only b == r//4, col-reduce
                nt16 = paux.tile([B, R], f32, tag="aux", name="nt16")
                nc.tensor.matmul(nt16, mm(xkT_sb[:, :B]), mm(qT_sb),
                                 start=True, stop=True)
                nc.vector.tensor_mul(ntm_sb, nt16, maskf_sb)

                def batch_scores(b):
                    kt = kts[b // 4]
                    ktb = kt[:, rsp * (b % 4) : rsp * (b % 4 + 1)]
                    rhs = mm(qT_sb[:, NH * b : NH * (b + 1)])
                    qk = pqk.tile([128, BCH * NH], f32, tag="qkb", name="qk")
                    for c in range(BCH):
                        nc.tensor.matmul(
                            qk[:, NH * c : NH * (c + 1)],
                            mm(ktb[:, 128 * c : 128 * (c + 1)]), rhs,
                            start=True, stop=True,
                        )
                    out_ap = sT_sb[:, SH_CH : SH_CH + BCH, NH * b : NH * (b + 1)]
                    nc.vector.tensor_copy(
                        out_ap, qk.rearrange("p (c n) -> p c n", n=NH)
                    )

                def exp_quarter(g):
                    # cols 16g..16g+16 of every chunk, incl shared + new-token
                    nc.scalar.activation(
                        out=pT_sb[:, :, 16 * g : 16 * (g + 1)],
                        in_=sT_sb[:, :, 16 * g : 16 * (g + 1)],
                        func=AF.Exp, scale=SOFTMAX_SCALE,
                    )

                for g in range(NKG):
                    for j in range(4):
                        batch_scores(4 * g + j)
                    if g == 0:
                        # reduce masked new-token scores into sT row 0
                        ntr = paux.tile([1, R], f32, tag="aux", name="ntr")
                        nc.tensor.matmul(ntr, onescf, ntm_sb,
                                         start=True, stop=True)
                        nc.vector.tensor_copy(sT_sb[0:1, NCH - 1, :], ntr)
                    exp_quarter(g)

                # ---------------- PV: v chunks stationary -> attnT ----------
                # one PSUM bank [128hd, 64r]; zero it once up front (per-slice
                # start=True would re-arm the whole 2KB bank), then accumulate
                pv = ppv.tile([128, R], f32)
                nc.tensor.matmul(pv, mm(ones1p), mm(zeros1p),
                                 start=True, stop=False,
                                 skip_group_check=True)
                for g in range(NKG):
                    vt = vts[g]
                    for j in range(4):
                        b = 4 * g + j
                        vb = vt[:, rsp * j : rsp * (j + 1)]
                        for c in range(BCH):
                            nc.tensor.matmul(
                                pv[:, NH * b : NH * (b + 1)],
                                mm(vb[:, 128 * c : 128 * (c + 1)]),
                                mm(pT_sb[:, SH_CH + c, NH * b : NH * (b + 1)]),
                                start=False, stop=False,
                                skip_group_check=True,
                            )

                # new-token PV: bcast p_new to 16 partitions, mask, then
                # xv.T @ (mask . p_new) accumulates [128, 64] in one matmul
                ntb = paux.tile([B, R], f32, tag="aux", name="ntb")
                nc.tensor.matmul(ntb, mm(ones1p[:, :B]),
                                 mm(pT_sb[0:1, NCH - 1, :]),
                                 start=True, stop=True)
                nc.vector.tensor_mul(ntp_sb, ntb, maskf_sb)
                nc.tensor.matmul(pv, mm(xv_sb), mm(ntp_sb),
                                 start=False, stop=False,
                                 skip_group_check=True)
                # shared-prefix PV, full width
                for c in range(SH_CH):
                    nc.tensor.matmul(
                        pv, mm(shv_sb[:, 128 * c : 128 * (c + 1)]),
                        mm(pT_sb[:, c, :]),
                        start=False, stop=(c == SH_CH - 1),
                        skip_group_check=True,
                    )
                nc.scalar.activation(out=attnT_sb, in_=pv, func=AF.Copy)

                # ---------------- rowsums + normalization ----------------
                s1 = ps1.tile([1, R], f32)
                for c in range(NCH):
                    nc.tensor.matmul(
                        s1, mm(ones_sb), mm(pT_sb[:, c, :]),
                        start=(c == 0), stop=(c == NCH - 1),
                    )
                nc.vector.tensor_copy(sum1_sb, s1)
                nc.vector.reciprocal(rinv1_sb, sum1_sb)
                rb = paux.tile([128, R], f32, tag="aux", name="rb")
                nc.tensor.matmul(rb, ones1pf, rinv1_sb, start=True, stop=True)
                nc.vector.tensor_copy(rinv_bc, rb)
                nc.vector.tensor_mul(attnTn_sb, attnT_sb, rinv_bc)
                # keep the PE ramped across the normalization chain so the
                # output projection runs at full p-state
                spt = paux.tile([64, R], DT, tag="aux", name="spt")
                for _ in range(10):
                    nc.tensor.transpose(spt, id_sb, id_sb)

            # ---------------- output projection ----------------
            attnH = attnTn_sb.rearrange("p (b h) -> p h b", h=NH)
            with tc.tile_pool(name="py", bufs=4, space="PSUM") as py:
                for n in range(8):
                    wot = wots[n // 2]
                    off = 4 * 512 * (n % 2)
                    y_ps = py.tile([B, 512], f32, tag="y", name="y_ps")
                    for g in range(4):
                        nc.tensor.matmul(
                            y_ps,
                            mm(attnH[:, g, :]),
                            mm(wot[:, off + 512 * g : off + 512 * (g + 1)]),
                            start=(g == 0), stop=(g == 3),
                        )
                    nc.vector.tensor_copy(
                        y_sb[:, 512 * n : 512 * (n + 1)], y_ps
                    )
                    nc.sync.dma_start(
                        out=y_d[:, 512 * n : 512 * (n + 1)],
                        in_=y_sb[:, 512 * n : 512 * (n + 1)],
                    )

    if os.environ.get("KERNEL_SKIP_LEGALIZE") != "1":
        _legalize_multiwait(nc)
    return nc


# ----------------------------------------------------------------------------
# host-side sharding / layout prep
# ----------------------------------------------------------------------------


def _np_dt(dt_name):
    if dt_name == "bfloat16":
        import ml_dtypes

        return ml_dtypes.bfloat16
    return np.float32


def _prep_inputs(inputs, spl, rsp, dt_name, kv_dt_name=None):
    nd = _np_dt(dt_name)
    from concourse import dt as _cdt, mybir as _mb
    nkv = _cdt.dt.np(getattr(_mb.dt, kv_dt_name)) if kv_dt_name else nd
    x = np.asarray(inputs["x"], np.float32)            # [16, 1, 4096]
    wq = np.asarray(inputs["wq"], np.float32)
    wk = np.asarray(inputs["wk"], np.float32)
    wv = np.asarray(inputs["wv"], np.float32)
    wo = np.asarray(inputs["wo"], np.float32)
    ck = np.asarray(inputs["cache_k"], np.float32)     # [16, 4096, 8, 128]
    cv = np.asarray(inputs["cache_v"], np.float32)
    shk = np.asarray(inputs["shared_cache_k"], np.float32)  # [1, 512, 8, 128]
    shv = np.asarray(inputs["shared_cache_v"], np.float32)
    cos = np.asarray(inputs["freqs_cos"], np.float32)  # [1, 64]
    sin = np.asarray(inputs["freqs_sin"], np.float32)

    xm = x[:, 0, :]                                    # [16, 4096]
    xT = np.ascontiguousarray(xm.T)                    # [4096, 16]
    xT_p = np.ascontiguousarray(
        xT.reshape(32, 128, B).transpose(1, 0, 2)
    ).reshape(128, 32 * B).astype(nd)

    # rope constants replicated over batch partitions; head-tiled for q;
    # plus the [16, 64] f32 mask (mask[b, r] = 1 iff r // 4 == b)
    crep = np.tile(cos.reshape(1, 1, 64), (B, NH, 1)).reshape(B, NH * 64)
    srep = np.tile(sin.reshape(1, 1, 64), (B, NH, 1)).reshape(B, NH * 64)
    mask = (np.arange(R)[None, :] // NH == np.arange(B)[:, None])
    ckrep = np.tile((cos / 64.0).reshape(1, 64), (B, 1))
    skrep = np.tile((sin / 64.0).reshape(1, 64), (B, 1))
    rpack = np.ascontiguousarray(
        np.concatenate(
            [crep, srep, mask.astype(np.float32), ckrep, skrep], axis=1
        ),
        np.float32,
    )                                                  # [16, 704]
    rpad = np.zeros((128, 2816), np.uint8)
    rpad[:B] = rpack.view(np.uint8).reshape(B, 2816)

    in_maps = []
    for m in range(N_CORES):
        wqm = wq[:, 512 * m : 512 * (m + 1)]           # [4096, 512]
        wq_p = np.ascontiguousarray(
            wqm.reshape(8, 4, 128, 512).transpose(0, 2, 1, 3)
        ).reshape(8, 128, 4 * 512).astype(nd)
        wkvm = np.concatenate(
            [wk[:, 128 * m : 128 * (m + 1)], wv[:, 128 * m : 128 * (m + 1)]],
            axis=1,
        )                                              # [4096, 256]
        wkv_p = np.ascontiguousarray(
            (wkvm * 64.0).reshape(32, 128, 256).transpose(1, 0, 2)
        ).reshape(128, 32 * 256).astype(nkv)
        wom = wo[512 * m : 512 * (m + 1), :]           # [512, 4096]
        wo_p = (
            wom.reshape(4, 128, 8, 512).transpose(2, 1, 0, 3)
        ).reshape(8, 128, 4 * 512)
        wo_p = np.ascontiguousarray(
            wo_p.reshape(4, 2, 128, 4 * 512).transpose(0, 2, 1, 3)
        ).reshape(4, 128, 2 * 4 * 512).astype(nd)

        # kT: [b, hd, j]; 4 batches side by side on the free dim
        ckm = ck[:, :rsp, m, :]                        # [16, rsp, 128]
        kT_p = np.ascontiguousarray(
            ckm.transpose(0, 2, 1).reshape(B // 4, 4, 128, rsp)
            .transpose(0, 2, 1, 3)
        ).reshape(B // 4, 128, 4 * rsp).astype(nkv)

        # v: partition-major [b, p, (c d)] with v[b, 128c+p, d] at [p, c, d]
        cvm = cv[:, :rsp, m, :]                        # [16, rsp, 128]
        v_pm = cvm.reshape(B, rsp // 128, 128, 128).transpose(0, 2, 1, 3)
        v_pm = v_pm.reshape(B, 128, rsp)
        v_p = np.ascontiguousarray(
            v_pm.reshape(B // 4, 4, 128, rsp).transpose(0, 2, 1, 3)
        ).reshape(B // 4, 128, 4 * rsp).astype(nkv)

        shkT_p = shk[0, :spl, m, :].T                  # [128 hd, spl]
        shv_p = (
            shv[0, :spl, m, :].reshape(spl // 128, 128, 128).transpose(1, 0, 2)
        ).reshape(128, spl)                            # [128 j, (c, hd)]
        sh_p = np.ascontiguousarray(
            np.concatenate([shkT_p, shv_p], axis=1)
        ).astype(nkv)

        pack = np.concatenate(
            [
                xT_p.view(np.uint8).reshape(128, 1024),
                sh_p.view(np.uint8).reshape(128, 1024),
                rpad,
            ],
            axis=1,
        )
        in_maps.append(
            {
                "pack": pack,
                "wq": wq_p,
                "wkv": wkv_p,
                "wo": wo_p,
                "kT": kT_p,
                "v": v_p,
            }
        )
    return in_maps


# ----------------------------------------------------------------------------
# entry point
# ----------------------------------------------------------------------------

_NC_CACHE = {}


def get_nc(spl=512, rsp=1536):
    key = (spl, rsp, STREAM_DTYPE, KV_DTYPE, F32R)
    if key not in _NC_CACHE:
        _patch_tile_drain()
        _install_ntff_hook()
        _NC_CACHE[key] = _build_nc(spl, rsp, STREAM_DTYPE, KV_DTYPE)
    return _NC_CACHE[key]


def prep_inputs(inputs):
    start_pos = int(inputs["start_pos"])
    spl = int(inputs["shared_prefix_length"])
    return _prep_inputs(inputs, spl, start_pos - spl, STREAM_DTYPE, KV_DTYPE)


def kernel(**inputs):
    from concourse.bass_utils import run_bass_kernel_spmd

    start_pos = int(inputs["start_pos"])
    spl = int(inputs["shared_prefix_length"])
    rsp = start_pos - spl
    nc = get_nc(spl, rsp)
    in_maps = _prep_inputs(inputs, spl, rsp, STREAM_DTYPE, KV_DTYPE)
    trace = os.environ.get("KERNEL_TRACE", "0") == "1"
    kwargs = {}
    if trace:
        kwargs = dict(
            trace=True,
            trace_cores=list(range(N_CORES)),
        )
    res = run_bass_kernel_spmd(
        nc, in_maps, core_ids=list(range(N_CORES)), **kwargs
    )
    kernel.last_result = res
    y = np.zeros((B, DIM), np.float64)
    for r in res.results:
        y += r["y"].astype(np.float64)
    return y.reshape(B, 1, DIM).astype(np.float32)



# revision 20
# speedup vs baseline: 1.1822x; 1.1822x over previous
"""Trainium2 Bass kernel for sparse (shared-prefix) GQA decode attention.

Full-input contract: kernel(**inputs) takes the unsharded tensors from
setup_inputs() and returns the full [16, 1, 4096] float32 output.

Sharding: tensor-parallel over heads across 8 NeuronCores. Core m owns
query heads 4m..4m+3 and kv head m (GQA group m), i.e. wq columns
[512m, 512m+512), wk/wv columns [128m, 128m+128), wo rows [512m, 512m+512),
and head m of the kv caches. Each core computes a partial output
y_m = attn_m @ wo_m; the host sums the 8 partials (the "all-reduce").

Device-side layout: scores are kept transposed, sT[j, r] with r = 4b+h on
the free dim, so every engine op starts at partition 0 (the hardware only
allows aligned partition bases) and the probabilities come out already in
the orientation the PV matmul needs.

Problem constants (hardcoded per the harness contract): bsz=16, seqlen=1,
dim=4096, n_heads=32, n_kv=8, hd=128, start_pos=2048,
shared_prefix_length=512 -> rsp=1536, L=2049.
"""

import math
import os
import sys
import types

import numpy as np

# ----------------------------------------------------------------------------
# environment patches (self-contained; no /root/problem reads)
# ----------------------------------------------------------------------------


def _patch_tile_drain():
    """The stock TileContext._drain_and_barrier puts one sem-wait per live
    semaphore on a single Drain instruction; the walrus build in this image
    only accepts a single sync wait per instruction ("Too many sync wait
    commands"). Re-emit the waits as individual EventSemaphore instructions
    on the same sequencer instead."""
    import concourse.tile as tile
    from concourse.vector_clock import ScopedClock

    if getattr(tile.TileContext, "_drain_patched", False):
        return

    def _drain_and_barrier(self, tick_clock, wait_clock):
        nc = self.nc
        drain_inst = nc.sync.drain()
        wait_clock.add_sem_waits(
            drain_inst.ins, ScopedClock({None: tick_clock.global_clock})
        )
        waits = list(drain_inst.ins.sync_info.on_wait)
        if len(waits) > 1:
            by_name = {h.name: h for h in self.sems.allocated().values()}
            try:
                drain_inst.ins.sync_info = None
            except Exception:
                pass
            for w in waits:
                h = by_name.get(w.ant_name)
                assert h is not None, f"no handle for sem {w.ant_name}"
                nc.sync.wait_ge(h, w.wait_value)

        # No barrier / explicit sem clears: every instruction transitively
        # precedes the SP wait chain above, and the NRT postamble already
        # resets all semaphores. Only do the python-side bookkeeping.
        assert self.sems is not None
        popped = nc._tile_sem_poison_stack.pop()
        assert popped is self._sem_poison
        nums = [h.num for h in self.sems.allocated().values()]
        nc._state.prepend_free_semaphores(nums)
        for ps in nc._tile_sem_poison_stack:
            ps.update(nums)

    tile.TileContext._drain_and_barrier = _drain_and_barrier
    tile.TileContext._drain_patched = True


def _install_ntff_hook():
    """Optional: register the axon NTFF profile hook (missing from the
    trimmed antenv package) so trace=True works for profiling, and stub the
    S3 artifact upload (zero-egress container)."""
    try:
        if "antenv.axon_hooks" not in sys.modules:
            mod = types.ModuleType("antenv.axon_hooks")
            mod._hook = None
            mod.set_axon_ntff_profile_hook = lambda h: setattr(mod, "_hook", h)
            mod.get_axon_ntff_profile_hook = lambda: mod._hook
            sys.modules["antenv.axon_hooks"] = mod
            import antenv

            antenv.axon_hooks = mod
            from trn_agent_boot.trn_boot import _ntff_profile_via_ctypes

            mod.set_axon_ntff_profile_hook(
                _ntff_profile_via_ctypes("/opt/axon/libaxon_pjrt.so")
            )
        import concourse.bass_utils as bu

        bu.upload_artifacts = lambda tmpdir: tmpdir
    except Exception:
        pass




def _legalize_multiwait(nc, max_waits=1):
    """This walrus build accepts at most one sync wait per instruction.
    Hoist excess waits into standalone single-wait EventSemaphore
    instructions inserted immediately before, on the same engine."""
    import bass_rust

    uid = 0
    for f in nc.m.functions:
        for bb in f.blocks:
            insts = list(bb.instructions)
            out = []
            changed = False
            for ins in insts:
                si = ins.sync_info
                if si is not None:
                    waits = list(si.on_wait)
                    if len(waits) > max_waits:
                        for w in waits[:-max_waits]:
                            ev = bass_rust.InstEventSemaphore(
                                name=f"{ins.name}_xw{uid}"
                            )
                            uid += 1
                            ev.engine = ins.engine
                            ev.sync_info = bass_rust.SyncInfo(
                                on_wait=[w], on_update=[]
                            )
                            out.append(ev)
                        ins.sync_info = bass_rust.SyncInfo(
                            on_wait=waits[-max_waits:],
                            on_update=list(si.on_update),
                        )
                        changed = True
                out.append(ins)
            if changed:
                bb.instructions = out


# ----------------------------------------------------------------------------
# constants
# ----------------------------------------------------------------------------

N_CORES = 8
B = 16            # batch
DIM = 4096
N_HEADS = 32
N_KV = 8
HD = 128
NH = N_HEADS // N_CORES      # 4 local q heads
R = B * NH                   # 64 (b,h) rows, r = 4*b + h
SOFTMAX_SCALE = 1.0 / math.sqrt(HD)
NEG_BIG = -1.0e30

# stream dtype for weights / kv-cache / matmul operands. "bfloat16" halves the
# HBM traffic (memory-bound kernel); softmax stays fp32 and all matmuls
# accumulate in fp32 PSUM.
STREAM_DTYPE = os.environ.get("KERNEL_STREAM_DTYPE", "bfloat16")
# kv-cache stream dtype: float8e3 (e3m4) halves the dominant HBM stream;
# scores/PV matmuls mix fp8 k/v with bf16 q/probs (PE allows mixed operands)
KV_DTYPE = os.environ.get("KERNEL_KV_DTYPE", "float8e3")
# use the fp32r (full-rate) matmul mode when streaming fp32
F32R = os.environ.get("KERNEL_F32R", "1") == "1"


# ----------------------------------------------------------------------------
# device kernel
# ----------------------------------------------------------------------------


def _build_nc(spl, rsp, dt_name, kv_dt_name):
    import concourse.bass as bass
    import concourse.tile as tile
    from concourse import mybir
    from concourse.masks import make_identity
    from concourse.mybir import ActivationFunctionType as AF

    DT = getattr(mybir.dt, dt_name)
    KVDT = getattr(mybir.dt, kv_dt_name)
    f32 = mybir.dt.float32
    assert spl % 128 == 0 and rsp % 512 == 0
    SH_CH = spl // 128          # shared j-chunks (4)
    BCH = rsp // 128            # per-batch j-chunks (12)
    NCH = SH_CH + BCH + 1       # total chunks incl. new-token chunk (17)
    NWQ = 8                     # wq split into 8 fine tiles (stream chasing)
    WQK = 32 // NWQ             # k-chunks per wq tile (4)
    NKG = 4                     # kv batch groups (4 batches each)
    SPIN = int(os.environ.get("KERNEL_SPIN", "56"))

    def mm(ap):
        if dt_name == "float32" and F32R:
            return ap.bitcast(mybir.dt.float32r)
        return ap

    nc = bass.Bass(
        "TRN2", target_bir_lowering=False, debug=False, num_devices=N_CORES
    )

    def din(name, shape, dt=DT):
        return nc.dram_tensor(name, shape, dt, kind="ExternalInput").ap()

    # byte-packed consts: xT (bf16, 1KB) | shared kT+v (fp8, 1KB) |
    # rope cos/sin + mask (f32, rows 0-15, 2304B)
    pack_d = din("pack", [128, 4864], mybir.dt.uint8)
    wq_d = din("wq", [NWQ, 128, WQK * 512])
    wkv_d = din("wkv", [128, 32 * 256], KVDT)
    kT_d = din("kT", [NKG, 128, 4 * rsp], KVDT)
    v_d = din("v", [NKG - 1, 128, 4 * rsp], KVDT)
    v3_d = din("v3", [2, 128, 2 * rsp], KVDT)
    wo_d = din("wo", [4, 128, 2 * 4 * 512])
    y_d = nc.dram_tensor("y", [B, DIM], f32, kind="ExternalOutput").ap()

    with tile.TileContext(nc) as tc:
        with tc.tile_pool(name="const", bufs=1) as const, \
             tc.tile_pool(name="wpool", bufs=NWQ) as wpool, \
             tc.tile_pool(name="kpool", bufs=NKG) as kpool, \
             tc.tile_pool(name="vpool", bufs=NKG + 1) as vpool, \
             tc.tile_pool(name="wopool", bufs=4) as wopool, \
             tc.tile_pool(name="tmp", bufs=4) as tmp:

            # ---------------- resident tiles ----------------
            id_sb = const.tile([64, 64], DT)
            make_identity(nc, id_sb)
            ones_sb = const.tile([128, 1], DT)
            nc.vector.memset(ones_sb, 1.0)
            ones1p = const.tile([1, 128], DT)
            nc.vector.memset(ones1p, 1.0)
            ones1pf = const.tile([1, 128], f32)
            nc.vector.memset(ones1pf, 1.0)
            onescf = const.tile([16, 1], f32)
            nc.vector.memset(onescf, 1.0)
            zeros1p = const.tile([1, R], DT)
            nc.vector.memset(zeros1p, 0.0)

            pack_sb = const.tile([128, 4864], mybir.dt.uint8)
            nc.sync.dma_start(out=pack_sb, in_=pack_d)
            xT_sb = pack_sb[:, :1024].bitcast(DT)
            shkT_sb = pack_sb[:, 1024 : 1024 + spl].bitcast(KVDT)
            shv_sb = pack_sb[:, 1024 + spl : 2048].bitcast(KVDT)
            rp_sb = pack_sb[:B, 2048:].bitcast(f32)    # [16, 704]
            crep_sb = rp_sb[:, : NH * 64]
            srep_sb = rp_sb[:, NH * 64 : 2 * NH * 64]
            maskf_sb = rp_sb[:, 2 * NH * 64 : 2 * NH * 64 + 64]  # b==r//4
            ckrep_sb = rp_sb[:, 2 * NH * 64 + 64 : 2 * NH * 64 + 128]  # cos/64
            skrep_sb = rp_sb[:, 2 * NH * 64 + 128 :]                   # sin/64
            mask_bf = const.tile([B, 64], DT)
            nc.vector.tensor_copy(mask_bf, maskf_sb)

            qT_sb = const.tile([128, R], DT)        # cols r = 4b+h
            xkT_sb = const.tile([128, B], DT)
            xv_sb = const.tile([B, HD], DT)
            sT_sb = const.tile([128, NCH, R], f32)  # transposed scores
            pT_sb = const.tile([128, NCH, R], DT)   # transposed probabilities
            sum1_sb = const.tile([1, R], f32)
            rinv1_sb = const.tile([1, R], f32)
            rinv_bc = const.tile([128, R], DT)      # rinv broadcast, cols r
            attnT_sb = const.tile([128, R], DT)     # PV result, cols r
            attnTn_sb = const.tile([128, R], DT)    # normalized
            ntm_sb = const.tile([B, R], f32)        # masked new-token scores
            ntp_sb = const.tile([B, R], DT)         # masked new-token probs
            y_sb = const.tile([B, DIM], f32)

            # ---------------- PE p-state warmup spin ----------------
            if SPIN:
                with tc.tile_pool(name="pwarm", bufs=1, space="PSUM") as pw:
                    wps = pw.tile([64, 64], DT)
                    for _ in range(SPIN):
                        nc.tensor.transpose(wps, id_sb, id_sb)

            # ---------------- phase A: projections + rope ----------------
            with tc.tile_pool(name="psA", bufs=1, space="PSUM") as psA, \
                 tc.tile_pool(name="ptrA", bufs=2, space="PSUM") as ptrA:
                xq_ps = psA.tile([B, NH * HD], f32)
                for t in range(NWQ):
                    wt = wpool.tile([128, WQK * 512], DT, tag="wq", name="wt")
                    nc.sync.dma_start(out=wt, in_=wq_d[t])
                    for c in range(WQK):
                        k = WQK * t + c
                        nc.tensor.matmul(
                            xq_ps,
                            mm(xT_sb[:, B * k : B * (k + 1)]),
                            mm(wt[:, 512 * c : 512 * (c + 1)]),
                            start=(k == 0),
                            stop=(k == 31),
                        )
                xkv_ps = psA.tile([B, 2 * HD], f32)
                wkv_sb = const.tile([128, 32 * 256], KVDT)
                nc.sync.dma_start(out=wkv_sb, in_=wkv_d)
                for k in range(32):
                    nc.tensor.matmul(
                        xkv_ps,
                        mm(xT_sb[:, B * k : B * (k + 1)]),
                        mm(wkv_sb[:, 256 * k : 256 * (k + 1)]),
                        start=(k == 0),
                        stop=(k == 31),
                    )

                # rope: pairs (even, odd) along hd; cos/sin repeated per
                # head (k uses cos/64, sin/64 to descale the x64 fp8 wkv)
                def rope(dst, src_ps, width, c_ap=None, s_ap=None):
                    e = src_ps.rearrange("p (n two) -> p n two", two=2)[:, :, 0]
                    o = src_ps.rearrange("p (n two) -> p n two", two=2)[:, :, 1]
                    de = dst.rearrange("p (n two) -> p n two", two=2)[:, :, 0]
                    do = dst.rearrange("p (n two) -> p n two", two=2)[:, :, 1]
                    c_ap = crep_sb[:, :width] if c_ap is None else c_ap
                    s_ap = srep_sb[:, :width] if s_ap is None else s_ap
                    t1 = tmp.tile([B, NH * 64], f32, tag="t1", name="t1")[:, :width]
                    t2 = tmp.tile([B, NH * 64], f32, tag="t2", name="t2")[:, :width]
                    nc.vector.tensor_mul(t1, e, c_ap)
                    nc.vector.tensor_mul(t2, o, s_ap)
                    nc.vector.tensor_sub(de, t1, t2)
                    t3 = tmp.tile([B, NH * 64], f32, tag="t1", name="t3")[:, :width]
                    t4 = tmp.tile([B, NH * 64], f32, tag="t2", name="t4")[:, :width]
                    nc.vector.tensor_mul(t3, e, s_ap)
                    nc.vector.tensor_mul(t4, o, c_ap)
                    nc.vector.tensor_add(do, t3, t4)

                xq_r = const.tile([B, NH * HD], DT)
                rope(xq_r, xq_ps, NH * 64)
                xk_r = const.tile([B, HD], DT)
                rope(xk_r, xkv_ps[:, :HD], 64, ckrep_sb, skrep_sb)
                nc.scalar.activation(out=xv_sb, in_=xkv_ps[:, HD:],
                                     func=AF.Copy, scale=1.0 / 64.0)

                # qT (cols r = 4b+h) via per-head PE transposes
                for h in range(NH):
                    tp = ptrA.tile([128, B], DT, tag="tq", name="tp")
                    nc.tensor.transpose(
                        tp, xq_r[:, HD * h : HD * (h + 1)], id_sb[:B, :B]
                    )
                    out_ap = qT_sb.rearrange("p (b h) -> p b h", h=NH)[:, :, h]
                    nc.vector.tensor_copy(out_ap, tp)
                tpk = ptrA.tile([128, B], DT, tag="tq", name="tpk")
                nc.tensor.transpose(tpk, xk_r, id_sb[:B, :B])
                nc.vector.tensor_copy(xkT_sb, tpk)

            # ---------------- phase B: transposed scores ----------------
            # new-token chunk: partitions 1.. never written -> -inf
            nc.vector.memset(sT_sb[:, NCH - 1, :], NEG_BIG)

            kts = [
                kpool.tile([128, 4 * rsp], KVDT, tag="kt", name="kt")
                for _ in range(NKG)
            ]
            vts = [
                vpool.tile([128, 4 * rsp], KVDT, tag="vt", name="vt")
                for _ in range(NKG - 1)
            ]
            vt3 = [
                vpool.tile([128, 2 * rsp], KVDT, tag="vt", name="vt3")
                for _ in range(2)
            ]
            wots = [
                wopool.tile([128, 2 * 4 * 512], DT, tag="wo", name="wot")
                for _ in range(4)
            ]
            # kv issue order interleaved so scores lead PV by one group,
            # then the wo stream last (needed only for the tail projection)
            for a, b in [(0, None), (1, 0), (2, 1), (3, 2)]:
                if a is not None:
                    nc.sync.dma_start(out=kts[a], in_=kT_d[a])
                if b is not None:
                    nc.sync.dma_start(out=vts[b], in_=v_d[b])
            # last group as two 2-batch transfers so the tail PV starts
            # as soon as the first half lands
            nc.sync.dma_start(out=vt3[0], in_=v3_d[0])
            nc.sync.dma_start(out=vt3[1], in_=v3_d[1])
            for n in range(4):
                nc.sync.dma_start(out=wots[n], in_=wo_d[n])

            with tc.tile_pool(name="pqk", bufs=4, space="PSUM") as pqk, \
                 tc.tile_pool(name="ppv", bufs=1, space="PSUM") as ppv, \
                 tc.tile_pool(name="ps1", bufs=1, space="PSUM") as ps1, \
                 tc.tile_pool(name="paux", bufs=2, space="PSUM") as paux:

                # shared-prefix scores: all 64 q rows at once per j-chunk
                for c in range(SH_CH):
                    qs = pqk.tile([128, R], f32, tag="qkb", name="qs")
                    nc.tensor.matmul(
                        qs, mm(shkT_sb[:, 128 * c : 128 * (c + 1)]), mm(qT_sb),
                        start=True, stop=True,
                    )
                    nc.vector.tensor_copy(sT_sb[:, c, :], qs)

                # new-token scores, all batches in one masked matmul:
                # nt16[b, r] = xk_b . q_r ; keep only b == r//4, col-reduce
                nt16 = paux.tile([B, R], f32, tag="aux", name="nt16")
                nc.tensor.matmul(nt16, mm(xkT_sb[:, :B]), mm(qT_sb),
                                 start=True, stop=True)
                nc.vector.tensor_mul(ntm_sb, nt16, maskf_sb)

                def batch_scores(b):
                    kt = kts[b // 4]
                    ktb = kt[:, rsp * (b % 4) : rsp * (b % 4 + 1)]
                    rhs = mm(qT_sb[:, NH * b : NH * (b + 1)])
                    qk = pqk.tile([128, BCH * NH], f32, tag="qkb", name="qk")
                    for c in range(BCH):
                        nc.tensor.matmul(
                            qk[:, NH * c : NH * (c + 1)],
                            mm(ktb[:, 128 * c : 128 * (c + 1)]), rhs,
                            start=True, stop=True,
                        )
                    out_ap = sT_sb[:, SH_CH : SH_CH + BCH, NH * b : NH * (b + 1)]
                    nc.vector.tensor_copy(
                        out_ap, qk.rearrange("p (c n) -> p c n", n=NH)
                    )

                def exp_quarter(g):
                    # cols 16g..16g+16 of every chunk, incl shared + new-token
                    nc.scalar.activation(
                        out=pT_sb[:, :, 16 * g : 16 * (g + 1)],
                        in_=sT_sb[:, :, 16 * g : 16 * (g + 1)],
                        func=AF.Exp, scale=SOFTMAX_SCALE,
                    )

                for g in range(NKG):
                    for j in range(4):
                        batch_scores(4 * g + j)
                    if g == 0:
                        # reduce masked new-token scores into sT row 0
                        ntr = paux.tile([1, R], f32, tag="aux", name="ntr")
                        nc.tensor.matmul(ntr, onescf, ntm_sb,
                                         start=True, stop=True)
                        nc.vector.tensor_copy(sT_sb[0:1, NCH - 1, :], ntr)
                    exp_quarter(g)

                # ---------------- PV: v chunks stationary -> attnT ----------
                # one PSUM bank [128hd, 64r]; zero it once up front (per-slice
                # start=True would re-arm the whole 2KB bank), then accumulate
                pv = ppv.tile([128, R], f32)
                nc.tensor.matmul(pv, mm(ones1p), mm(zeros1p),
                                 start=True, stop=False,
                                 skip_group_check=True)
                for g in range(NKG):
                    for j in range(4):
                        b = 4 * g + j
                        if g == NKG - 1:
                            vb = vt3[j // 2][
                                :, rsp * (j % 2) : rsp * (j % 2 + 1)
                            ]
                        else:
                            vb = vts[g][:, rsp * j : rsp * (j + 1)]
                        for c in range(BCH):
                            nc.tensor.matmul(
                                pv[:, NH * b : NH * (b + 1)],
                                mm(vb[:, 128 * c : 128 * (c + 1)]),
                                mm(pT_sb[:, SH_CH + c, NH * b : NH * (b + 1)]),
                                start=False, stop=False,
                                skip_group_check=True,
                            )

                # new-token PV: bcast p_new to 16 partitions, mask, then
                # xv.T @ (mask . p_new) accumulates [128, 64] in one matmul
                ntb = paux.tile([B, R], f32, tag="aux", name="ntb")
                nc.tensor.matmul(ntb, mm(ones1p[:, :B]),
                                 mm(pT_sb[0:1, NCH - 1, :]),
                                 start=True, stop=True)
                nc.vector.tensor_mul(ntp_sb, ntb, maskf_sb)
                nc.tensor.matmul(pv, mm(xv_sb), mm(ntp_sb),
                                 start=False, stop=False,
                                 skip_group_check=True)
                # shared-prefix PV, full width
                for c in range(SH_CH):
                    nc.tensor.matmul(
                        pv, mm(shv_sb[:, 128 * c : 128 * (c + 1)]),
                        mm(pT_sb[:, c, :]),
                        start=False, stop=(c == SH_CH - 1),
                        skip_group_check=True,
                    )
                nc.scalar.activation(out=attnT_sb, in_=pv, func=AF.Copy)

                # ---------------- rowsums + normalization ----------------
                s1 = ps1.tile([1, R], f32)
                for c in range(NCH):
                    nc.tensor.matmul(
                        s1, mm(ones_sb), mm(pT_sb[:, c, :]),
                        start=(c == 0), stop=(c == NCH - 1),
                    )
                nc.vector.tensor_copy(sum1_sb, s1)
                nc.vector.reciprocal(rinv1_sb, sum1_sb)
                rb = paux.tile([128, R], f32, tag="aux", name="rb")
                nc.tensor.matmul(rb, ones1pf, rinv1_sb, start=True, stop=True)
                nc.vector.tensor_copy(rinv_bc, rb)
                nc.vector.tensor_mul(attnTn_sb, attnT_sb, rinv_bc)
                # keep the PE ramped across the normalization chain so the
                # output projection runs at full p-state
                spt = paux.tile([64, R], DT, tag="aux", name="spt")
                for _ in range(10):
                    nc.tensor.transpose(spt, id_sb, id_sb)

            # ---------------- output projection ----------------
            attnH = attnTn_sb.rearrange("p (b h) -> p h b", h=NH)
            with tc.tile_pool(name="py", bufs=4, space="PSUM") as py:
                for n in range(8):
                    wot = wots[n // 2]
                    off = 4 * 512 * (n % 2)
                    y_ps = py.tile([B, 512], f32, tag="y", name="y_ps")
                    for g in range(4):
                        nc.tensor.matmul(
                            y_ps,
                            mm(attnH[:, g, :]),
                            mm(wot[:, off + 512 * g : off + 512 * (g + 1)]),
                            start=(g == 0), stop=(g == 3),
                        )
                    nc.vector.tensor_copy(
                        y_sb[:, 512 * n : 512 * (n + 1)], y_ps
                    )
                    nc.sync.dma_start(
                        out=y_d[:, 512 * n : 512 * (n + 1)],
                        in_=y_sb[:, 512 * n : 512 * (n + 1)],
                    )

    if os.environ.get("KERNEL_SKIP_LEGALIZE") != "1":
        _legalize_multiwait(nc)
    return nc


# ----------------------------------------------------------------------------
# host-side sharding / layout prep
# ----------------------------------------------------------------------------


def _np_dt(dt_name):
    if dt_name == "bfloat16":
        import ml_dtypes

        return ml_dtypes.bfloat16
    return np.float32


def _prep_inputs(inputs, spl, rsp, dt_name, kv_dt_name=None):
    nd = _np_dt(dt_name)
    from concourse import dt as _cdt, mybir as _mb
    nkv = _cdt.dt.np(getattr(_mb.dt, kv_dt_name)) if kv_dt_name else nd
    x = np.asarray(inputs["x"], np.float32)            # [16, 1, 4096]
    wq = np.asarray(inputs["wq"], np.float32)
    wk = np.asarray(inputs["wk"], np.float32)
    wv = np.asarray(inputs["wv"], np.float32)
    wo = np.asarray(inputs["wo"], np.float32)
    ck = np.asarray(inputs["cache_k"], np.float32)     # [16, 4096, 8, 128]
    cv = np.asarray(inputs["cache_v"], np.float32)
    shk = np.asarray(inputs["shared_cache_k"], np.float32)  # [1, 512, 8, 128]
    shv = np.asarray(inputs["shared_cache_v"], np.float32)
    cos = np.asarray(inputs["freqs_cos"], np.float32)  # [1, 64]
    sin = np.asarray(inputs["freqs_sin"], np.float32)

    xm = x[:, 0, :]                                    # [16, 4096]
    xT = np.ascontiguousarray(xm.T)                    # [4096, 16]
    xT_p = np.ascontiguousarray(
        xT.reshape(32, 128, B).transpose(1, 0, 2)
    ).reshape(128, 32 * B).astype(nd)

    # rope constants replicated over batch partitions; head-tiled for q;
    # plus the [16, 64] f32 mask (mask[b, r] = 1 iff r // 4 == b)
    crep = np.tile(cos.reshape(1, 1, 64), (B, NH, 1)).reshape(B, NH * 64)
    srep = np.tile(sin.reshape(1, 1, 64), (B, NH, 1)).reshape(B, NH * 64)
    mask = (np.arange(R)[None, :] // NH == np.arange(B)[:, None])
    ckrep = np.tile((cos / 64.0).reshape(1, 64), (B, 1))
    skrep = np.tile((sin / 64.0).reshape(1, 64), (B, 1))
    rpack = np.ascontiguousarray(
        np.concatenate(
            [crep, srep, mask.astype(np.float32), ckrep, skrep], axis=1
        ),
        np.float32,
    )                                                  # [16, 704]
    rpad = np.zeros((128, 2816), np.uint8)
    rpad[:B] = rpack.view(np.uint8).reshape(B, 2816)

    in_maps = []
    for m in range(N_CORES):
        wqm = wq[:, 512 * m : 512 * (m + 1)]           # [4096, 512]
        wq_p = np.ascontiguousarray(
            wqm.reshape(8, 4, 128, 512).transpose(0, 2, 1, 3)
        ).reshape(8, 128, 4 * 512).astype(nd)
        wkvm = np.concatenate(
            [wk[:, 128 * m : 128 * (m + 1)], wv[:, 128 * m : 128 * (m + 1)]],
            axis=1,
        )                                              # [4096, 256]
        wkv_p = np.ascontiguousarray(
            (wkvm * 64.0).reshape(32, 128, 256).transpose(1, 0, 2)
        ).reshape(128, 32 * 256).astype(nkv)
        wom = wo[512 * m : 512 * (m + 1), :]           # [512, 4096]
        wo_p = (
            wom.reshape(4, 128, 8, 512).transpose(2, 1, 0, 3)
        ).reshape(8, 128, 4 * 512)
        wo_p = np.ascontiguousarray(
            wo_p.reshape(4, 2, 128, 4 * 512).transpose(0, 2, 1, 3)
        ).reshape(4, 128, 2 * 4 * 512).astype(nd)

        # kT: [b, hd, j]; 4 batches side by side on the free dim
        ckm = ck[:, :rsp, m, :]                        # [16, rsp, 128]
        kT_p = np.ascontiguousarray(
            ckm.transpose(0, 2, 1).reshape(B // 4, 4, 128, rsp)
            .transpose(0, 2, 1, 3)
        ).reshape(B // 4, 128, 4 * rsp).astype(nkv)

        # v: partition-major [b, p, (c d)] with v[b, 128c+p, d] at [p, c, d]
        cvm = cv[:, :rsp, m, :]                        # [16, rsp, 128]
        v_pm = cvm.reshape(B, rsp // 128, 128, 128).transpose(0, 2, 1, 3)
        v_pm = v_pm.reshape(B, 128, rsp)
        v_p4 = np.ascontiguousarray(
            v_pm.reshape(B // 4, 4, 128, rsp).transpose(0, 2, 1, 3)
        ).reshape(B // 4, 128, 4 * rsp).astype(nkv)
        v_p = np.ascontiguousarray(v_p4[: B // 4 - 1])
        v3_p = np.ascontiguousarray(
            v_p4[B // 4 - 1].reshape(128, 2, 2 * rsp).transpose(1, 0, 2)
        )

        shkT_p = shk[0, :spl, m, :].T                  # [128 hd, spl]
        shv_p = (
            shv[0, :spl, m, :].reshape(spl // 128, 128, 128).transpose(1, 0, 2)
        ).reshape(128, spl)                            # [128 j, (c, hd)]
        sh_p = np.ascontiguousarray(
            np.concatenate([shkT_p, shv_p], axis=1)
        ).astype(nkv)

        pack = np.concatenate(
            [
                xT_p.view(np.uint8).reshape(128, 1024),
                sh_p.view(np.uint8).reshape(128, 1024),
                rpad,
            ],
            axis=1,
        )
        in_maps.append(
            {
                "pack": pack,
                "wq": wq_p,
                "wkv": wkv_p,
                "wo": wo_p,
                "kT": kT_p,
                "v": v_p,
                "v3": v3_p,
            }
        )
    return in_maps


# ----------------------------------------------------------------------------
# entry point
# ----------------------------------------------------------------------------

_NC_CACHE = {}


def get_nc(spl=512, rsp=1536):
    key = (spl, rsp, STREAM_DTYPE, KV_DTYPE, F32R)
    if key not in _NC_CACHE:
        _patch_tile_drain()
        _install_ntff_hook()
        _NC_CACHE[key] = _build_nc(spl, rsp, STREAM_DTYPE, KV_DTYPE)
    return _NC_CACHE[key]


def prep_inputs(inputs):
    start_pos = int(inputs["start_pos"])
    spl = int(inputs["shared_prefix_length"])
    return _prep_inputs(inputs, spl, start_pos - spl, STREAM_DTYPE, KV_DTYPE)


def kernel(**inputs):
    from concourse.bass_utils import run_bass_kernel_spmd

    start_pos = int(inputs["start_pos"])
    spl = int(inputs["shared_prefix_length"])
    rsp = start_pos - spl
    nc = get_nc(spl, rsp)
    in_maps = _prep_inputs(inputs, spl, rsp, STREAM_DTYPE, KV_DTYPE)
    trace = os.environ.get("KERNEL_TRACE", "0") == "1"
    kwargs = {}
    if trace:
        kwargs = dict(
            trace=True,
            trace_cores=list(range(N_CORES)),
        )
    res = run_bass_kernel_spmd(
        nc, in_maps, core_ids=list(range(N_CORES)), **kwargs
    )
    kernel.last_result = res
    y = np.zeros((B, DIM), np.float64)
    for r in res.results:
        y += r["y"].astype(np.float64)
    return y.reshape(B, 1, DIM).astype(np.float32)

